# revision 18
# baseline (speedup 1.0000x reference)
"""Data-adaptive weight-ensembling MLP (per-sample expert-merged FFN) on 8 trn2 cores.

v4: fp8(E3M4) task-vector banks + 4-way column-tiled PE accumulation +
layer-2 resharded by output columns so NO final cross-core reduction is
needed (host-side concat of disjoint j-slices). The only collective is a
16KB AllGather of the layer-1 activations, fully hidden under the tv2
DMA stream.

Math (per sample b):
  c[b,:,:]  = gate(x)[b].reshape(E, L)          (2-layer relu MLP gate)
  W1[b] = bW1 + sum_e c[b,e,0] tvW1[e];  b1[b] = bb1 + sum_e c[b,e,1] tvb1[e]
  W2[b] = bW2 + sum_e c[b,e,2] tvW2[e];  b2[b] = bb2 + sum_e c[b,e,3] tvb2[e]
  out[b] = relu(x[b] @ W1[b].T + b1[b]) @ W2[b].T + b2[b]

Merged weights are never materialized: expert matmuls stream the fp8 tv
banks through the PE as the moving operand (lhsT = per-expert scaled x,
bf16) and accumulate into one PSUM bank split into 4 column-strips
(partitions 32j..32j+15, tile_position=(0,32j)); a "sel" matmul (entries
1/STV at rows {b,32+b,64+b,96+b}) folds the strip sum AND the fp8 scale
compensation into one PE op.

Sharding (8 cores):
  L1: DFF split 8x512 -> exact local pre-activation + relu -> h1 [16,512].
  h1 transposed locally, AllGathered (16KB bf16) -> full h1T on all cores.
  L2: output D=1024 split 8x128 -> each core contracts the FULL DFF for
      its j-slice (tvW2 bank sharded by output column) -> exact out
      [16,128] per core, host-side np.concatenate. No AllReduce.
Task-vector banks staged host-side as E3M4 * 128 (clipped to +-15.5);
base weights bf16 * 128; gate W1/W2 as E3M4 * 512 (1/512 folded into the
relu / codings-copy activation scale). The base-weight pass runs FIRST in
layer 1 (depends only on xT, so the PE starts before the gate resolves)
and LAST in layer 2 (minimal tail after the final DMA lands).
"""

import contextlib

import numpy as np

B, D, DFF, E, L = 16, 1024, 4096, 16, 4
NCORES = 8
OSL = DFF // NCORES          # 512: per-core L1 DFF slice
OJ = D // NCORES             # 128: per-core L2 output-column slice
KC1 = D // 128               # 8 k-chunks for the layer-1 d contraction
KC2 = OSL // 128             # 4 local h1T chunks
KCF = DFF // 128             # 32 k-chunks for the layer-2 f contraction
NSTRIP = 4                   # column-strips in the PE array
EQUAD = E // 4               # experts are DMA'd in quads (2MB transfers)
STV = 128.0                  # fp8 tv-bank scale, folded out via sel=1/STV
GS = 512.0                   # fp8 gate-weight scale, folded into relu scale

_cache = {}


def _build(reps: int = 1, collective: bool = True, cfg: str = "mx8"):
    import concourse.bacc as bacc
    import concourse.bass as bass
    import concourse.tile as tile
    import concourse.mybir as mybir
    from concourse.masks import make_identity

    f32 = mybir.dt.float32
    bf16 = mybir.dt.bfloat16
    f8 = mybir.dt.float8e3
    # tvdt: dtype of the big fp8 banks; wdt: dtype of lhsT operands
    if cfg == "mx8":
        tvdt, wdt = f8, bf16
    elif cfg == "bf16":
        tvdt, wdt = bf16, bf16
    else:
        raise ValueError(cfg)
    Relu = mybir.ActivationFunctionType.Relu
    Copy = mybir.ActivationFunctionType.Copy
    nc = bacc.Bacc("TRN2", target_bir_lowering=False, debug=False,
                   num_devices=NCORES)

    # ---- I/O (per-core data supplied via in_maps) ----
    xT_h = nc.dram_tensor("xT", [128, KC1, B], wdt, kind="ExternalInput")
    gw1_h = nc.dram_tensor("gw1", [128, KC1, D], tvdt, kind="ExternalInput")
    gb1_h = nc.dram_tensor("gb1v", [1, D], f32, kind="ExternalInput")
    gw2_h = nc.dram_tensor("gw2", [128, KC1, E * L], tvdt,
                           kind="ExternalInput")
    gb2_h = nc.dram_tensor("gb2v", [1, E * L], f32, kind="ExternalInput")
    tv1_h = nc.dram_tensor("tv1", [EQUAD - 1, 128, 4, KC1, OSL], tvdt,
                           kind="ExternalInput")
    tv1b_h = nc.dram_tensor("tv1b", [2, 128, 2, KC1, OSL], tvdt,
                            kind="ExternalInput")
    bw1_h = nc.dram_tensor("bw1", [128, KC1, OSL], wdt, kind="ExternalInput")
    bb1_h = nc.dram_tensor("bb1v", [1, OSL], f32, kind="ExternalInput")
    tvb1_h = nc.dram_tensor("tvb1", [E, OSL], f32, kind="ExternalInput")
    tv2_h = nc.dram_tensor("tv2", [EQUAD, 128, 4, KCF, OJ], tvdt,
                           kind="ExternalInput")
    bw2_h = nc.dram_tensor("bw2", [128, KCF, OJ], wdt, kind="ExternalInput")
    bb2_h = nc.dram_tensor("bb2v", [1, OJ], f32, kind="ExternalInput")
    tvb2_h = nc.dram_tensor("tvb2", [E, OJ], f32, kind="ExternalInput")
    sel_h = nc.dram_tensor("sel", [128, B], wdt, kind="ExternalInput")
    out_h = nc.dram_tensor("out", [B, OJ], f32, kind="ExternalOutput")

    hg_in = nc.dram_tensor("hg_in", [128, KC2, B], wdt, kind="Internal")
    hg_out = nc.dram_tensor("hg_out", [NCORES, 128, KC2, B], wdt,
                            kind="Internal", addr_space="Shared")

    with tile.TileContext(nc) as tc, contextlib.ExitStack() as ctx:
        const = ctx.enter_context(tc.tile_pool(name="const", bufs=1))
        small = ctx.enter_context(tc.tile_pool(name="small", bufs=1))
        gwp = ctx.enter_context(tc.tile_pool(name="gwp", bufs=1))
        tvp1 = ctx.enter_context(tc.tile_pool(name="tvp1", bufs=3))
        tvp2 = ctx.enter_context(tc.tile_pool(name="tvp2", bufs=4))
        bwp = ctx.enter_context(tc.tile_pool(name="bwp", bufs=1))
        pacc = ctx.enter_context(tc.tile_pool(name="pacc", bufs=1,
                                              space="PSUM"))
        psml = ctx.enter_context(tc.tile_pool(name="psml", bufs=2,
                                              space="PSUM"))

        # constants (once)
        ones1 = const.tile([1, B], f32)
        nc.vector.memset(ones1[:], 1.0)
        ident16 = const.tile([B, B], f32)
        make_identity(nc, ident16[:])
        ones16_128 = const.tile([B, 128], f32)
        nc.vector.memset(ones16_128[:], 1.0)

        for _rep in range(reps):
            # small inputs
            xT = small.tile([128, KC1, B], wdt, name=f"xT_{_rep}", tag="xT")
            nc.sync.dma_start(out=xT[:], in_=xT_h.ap())
            selt = small.tile([128, B], wdt, name=f"sel_{_rep}", tag="sel")
            nc.sync.dma_start(out=selt[:], in_=sel_h.ap())
            gb1v = small.tile([1, D], f32, name=f"gb1v_{_rep}", tag="gb1v")
            nc.sync.dma_start(out=gb1v[:], in_=gb1_h.ap())
            gb2v = small.tile([1, E * L], f32, name=f"gb2v_{_rep}", tag="gb2v")
            nc.sync.dma_start(out=gb2v[:], in_=gb2_h.ap())
            bb1v = small.tile([1, OSL], f32, name=f"bb1v_{_rep}", tag="bb1v")
            nc.sync.dma_start(out=bb1v[:], in_=bb1_h.ap())
            tvb1t = small.tile([E, OSL], f32, name=f"tvb1t_{_rep}", tag="tvb1t")
            nc.sync.dma_start(out=tvb1t[:], in_=tvb1_h.ap())
            bb2v = small.tile([1, OJ], f32, name=f"bb2v_{_rep}", tag="bb2v")
            nc.sync.dma_start(out=bb2v[:], in_=bb2_h.ap())
            tvb2t = small.tile([E, OJ], f32, name=f"tvb2t_{_rep}", tag="tvb2t")
            nc.sync.dma_start(out=tvb2t[:], in_=tvb2_h.ap())
            gw2t = small.tile([128, KC1, E * L], tvdt, name=f"gw2t_{_rep}",
                              tag="gw2t")
            nc.sync.dma_start(out=gw2t[:], in_=gw2_h.ap())
            gw1t = gwp.tile([128, KC1, D], tvdt, name=f"gw1t_{_rep}",
                            tag="gw1t")
            nc.sync.dma_start(out=gw1t[:], in_=gw1_h.ap())

            gsc = 1.0 / GS if cfg == "mx8" else 1.0
            # ---- gate layer 1: g_h = relu((x @ gW1q.T + gb1*GS) / GS) ----
            # two 512-col halves on separate column-strips of one PSUM bank
            g_h = small.tile([B, D], f32, name=f"g_h_{_rep}", tag="g_h")
            gps = pacc.tile([128, 512], f32, tag="bank0")
            for n in range(2):
                nc.tensor.matmul(gps[32 * n:32 * n + B, :], ones1[:],
                                 gb1v[:, n * 512:(n + 1) * 512],
                                 start=True, stop=False,
                                 tile_position=(0, 32 * n))
            for kc in range(KC1):
                for n in range(2):
                    nc.tensor.matmul(gps[32 * n:32 * n + B, :], xT[:, kc, :],
                                     gw1t[:, kc, n * 512:(n + 1) * 512],
                                     start=False, stop=(kc == KC1 - 1),
                                     tile_position=(0, 32 * n))
            for n in range(2):
                nc.scalar.activation(g_h[:, n * 512:(n + 1) * 512],
                                     gps[32 * n:32 * n + B, :], Relu,
                                     scale=gsc)

            # ---- transpose g_h -> ghT [128, (kc, b)] ----
            ghT = small.tile([128, KC1, B], wdt, name=f"ghT_{_rep}", tag="ghT")
            for kc in range(KC1):
                pt = psml.tile([128, B], f32, tag="ps")
                nc.tensor.transpose(pt[:], g_h[:, kc * 128:(kc + 1) * 128],
                                    ident16[:])
                nc.vector.tensor_copy(ghT[:, kc, :], pt[:])

            # ---- gate layer 2: codings; cod[b, e, l] ----
            cps = pacc.tile([B, E * L], f32, tag="cps")
            nc.tensor.matmul(cps[:], ones1[:], gb2v[:], start=True, stop=False)
            for kc in range(KC1):
                nc.tensor.matmul(cps[:], ghT[:, kc, :], gw2t[:, kc, :],
                                 start=False, stop=(kc == KC1 - 1))
            cod = small.tile([B, E, L], f32, name=f"cod_{_rep}", tag="cod")
            nc.scalar.activation(cod[:],
                                 cps[:].rearrange("b (e l) -> b e l", e=E),
                                 Copy, scale=gsc)

            # ---- bias-coefficient matrices cT_l[e, b] = c[b, e, l] ----
            cT = {}
            for l in (1, 3):
                cl = small.tile([B, E], f32, name=f"cl{l}_{_rep}",
                                tag=f"cl{l}")
                nc.vector.tensor_copy(cl[:], cod[:, :, l])
                ptc = psml.tile([B, E], f32, tag="ps")
                nc.tensor.transpose(ptc[:], cl[:], ident16[:])
                cTl = small.tile([E, B], f32, name=f"cT{l}_{_rep}",
                                 tag=f"cT{l}")
                nc.vector.tensor_copy(cTl[:], ptc[:])
                cT[l] = cTl

            # ---- broadcast tiles cbc[l][e][p, b] = c[b, e, l] ----
            cbc = {0: [], 2: []}
            for l in (0, 2):
                for e in range(E):
                    diag = small.tile([B, B], f32, name=f"dg{l}_{e}_{_rep}",
                                      tag="diag")
                    nc.vector.tensor_scalar_mul(diag[:], ident16[:],
                                                cod[:, e, l:l + 1])
                    pb = psml.tile([128, B], f32, tag="ps")
                    nc.tensor.matmul(pb[:], ones16_128[:], diag[:],
                                     start=True, stop=True)
                    bc = small.tile([128, B], wdt, name=f"bc{l}_{e}_{_rep}",
                                    tag=f"bc{l}_{e}")
                    nc.vector.tensor_copy(bc[:], pb[:])
                    cbc[l].append(bc)

            # ---- X1T[e][128, kc, b] = xT * c0[b, e] ----
            x1t = []
            for e in range(E):
                t = small.tile([128, KC1, B], wdt, name=f"x1t{e}_{_rep}",
                               tag=f"x1t{e}")
                nc.vector.tensor_mul(
                    t[:], xT[:],
                    cbc[0][e][:, None, :].broadcast_to([128, KC1, B]))
                x1t.append(t)

            # ---- layer 1 expert+base matmuls, 4 column-strips ----
            ps1 = pacc.tile([128, 512], f32, tag="bank1")
            # zero the bank: rows the strip MMs never touch are multiplied
            # by sel's zeros, and stale non-finite PSUM would give 0*NaN=NaN
            nc.vector.memset(ps1[:], 0.0)
            idx = 0
            NMM1 = (E + 1) * KC1

            def strip_flags(idx, nmm):
                s = idx % NSTRIP
                return s, idx < NSTRIP, idx >= nmm - NSTRIP

            for pe_ in ["base", 0, 1, 2, "p0", "p1"]:
                if pe_ == "base":
                    tvt = bwp.tile([128, KC1, OSL], wdt, tag="bw1")
                    nc.sync.dma_start(out=tvt[:], in_=bw1_h.ap())
                    sub = [(xT, tvt[:])]
                elif isinstance(pe_, int):
                    tvt = tvp1.tile([128, 4, KC1, OSL], tvdt, tag="tv1")
                    nc.sync.dma_start(out=tvt[:], in_=tv1_h.ap()[pe_])
                    sub = [(x1t[4 * pe_ + i], tvt[:, i]) for i in range(4)]
                else:
                    # last quad split into two 1MB pairs: halves the MM tail
                    # between the final tv1 byte landing and the AllGather
                    pi = int(pe_[1])
                    tvt = bwp.tile([128, 2, KC1, OSL], tvdt,
                                   name=f"tv1b{pi}_{_rep}", tag=f"tv1b{pi}")
                    nc.sync.dma_start(out=tvt[:], in_=tv1b_h.ap()[pi])
                    sub = [(x1t[12 + 2 * pi + i], tvt[:, i]) for i in range(2)]
                for lhs, rv in sub:
                    for kc in range(KC1):
                        s, first, last = strip_flags(idx, NMM1)
                        nc.tensor.matmul(ps1[32 * s:32 * s + B, :],
                                         lhs[:, kc, :], rv[:, kc, :],
                                         start=first, stop=last,
                                         tile_position=(0, 32 * s))
                        idx += 1

            # ---- issue all layer-2 bank DMAs (SP FIFO stays saturated) ----
            tv2_tiles = []
            for pe_ in range(EQUAD):
                tvt2 = tvp2.tile([128, 4, KCF, OJ], tvdt,
                                 name=f"tv2_{pe_}_{_rep}", tag="tv2")
                nc.sync.dma_start(out=tvt2[:], in_=tv2_h.ap()[pe_])
                tv2_tiles.append(tvt2)
            bwt2 = bwp.tile([128, KCF, OJ], wdt, name=f"bw2_{_rep}", tag="bw2")
            nc.sync.dma_start(out=bwt2[:], in_=bw2_h.ap())
            tv2_tiles.append(bwt2)

            # ---- strip-sum + bias + relu: h1 = relu(sel.T@s1 + biases) ----
            s1 = small.tile([128, OSL], wdt, name=f"s1_{_rep}", tag="s1")
            nc.vector.tensor_copy(s1[:], ps1[:])
            ph1 = pacc.tile([B, 512], f32, tag="ph1")
            nc.tensor.matmul(ph1[:], ones1[:], bb1v[:], start=True, stop=False)
            nc.tensor.matmul(ph1[:], cT[1][:], tvb1t[:], start=False,
                             stop=False)
            nc.tensor.matmul(ph1[:], selt[:], s1[:], start=False, stop=True)
            h1 = small.tile([B, OSL], f32, name=f"h1_{_rep}", tag="h1")
            # per-chunk relu so each h1 transpose starts as soon as its
            # 128-column chunk is ready (pipelines ACT with PE transposes)
            for fc in range(KC2):
                nc.scalar.activation(h1[:, fc * 128:(fc + 1) * 128],
                                     ph1[:, fc * 128:(fc + 1) * 128], Relu)

            # ---- transpose h1 -> local h1T [128, (fc, b)] ----
            h1T = small.tile([128, KC2, B], wdt, name=f"h1T_{_rep}", tag="h1T")
            for fc in range(KC2):
                pt2 = psml.tile([128, B], f32, tag="ps")
                nc.tensor.transpose(pt2[:], h1[:, fc * 128:(fc + 1) * 128],
                                    ident16[:])
                nc.vector.tensor_copy(h1T[:, fc, :], pt2[:])

            # ---- AllGather h1T across cores -> full h1T [128, (r,fc), b] --
            nc.scalar.dma_start(out=hg_in.ap(), in_=h1T[:])
            nc.gpsimd.collective_compute(
                "AllGather", mybir.AluOpType.bypass,
                replica_groups=[list(range(NCORES))],
                ins=[hg_in.ap().opt()],
                outs=[hg_out.ap().opt()],
            )
            h1Tf = small.tile([128, NCORES, KC2, B], wdt,
                              name=f"h1Tf_{_rep}", tag="h1Tf")
            nc.scalar.dma_start(
                out=h1Tf[:],
                in_=hg_out.ap().rearrange("r p f b -> p r f b"))
            h1Tfv = h1Tf[:].rearrange("p r f b -> p (r f) b")

            # ---- X2T[e][128, (r,fc), b] = h1Tf * c2[b, e] ----
            x2t = []
            for e in range(E):
                t = small.tile([128, KCF, B], wdt, name=f"x2t{e}_{_rep}",
                               tag=f"x2t{e}")
                nc.vector.tensor_mul(
                    t[:], h1Tfv,
                    cbc[2][e][:, None, :].broadcast_to([128, KCF, B]))
                x2t.append(t)

            # ---- layer 2: full-DFF contraction for local j-slice ----
            ps2 = pacc.tile([128, OJ], f32, tag="bank2")
            nc.vector.memset(ps2[:], 0.0)
            idx2 = 0
            NMM2 = (E + 1) * KCF
            for pe_ in range(EQUAD + 1):
                if pe_ == EQUAD:
                    tvt2 = tv2_tiles[pe_]
                    sub = [(None, tvt2[:])]
                else:
                    tvt2 = tv2_tiles[pe_]
                    sub = [(x2t[4 * pe_ + i], tvt2[:, i]) for i in range(4)]
                for lhs, rv in sub:
                    for fc in range(KCF):
                        s, first, last = strip_flags(idx2, NMM2)
                        lap = (h1Tf[:, fc // KC2, fc % KC2, :]
                               if lhs is None else lhs[:, fc, :])
                        nc.tensor.matmul(ps2[32 * s:32 * s + B, :],
                                         lap, rv[:, fc, :],
                                         start=first, stop=last,
                                         tile_position=(0, 32 * s))
                        idx2 += 1

            # ---- strip-sum + bias; local j-slice is exact ----
            s2 = small.tile([128, OJ], wdt, name=f"s2_{_rep}", tag="s2")
            nc.vector.tensor_copy(s2[:], ps2[:])
            po = pacc.tile([B, OJ], f32, tag="po")
            nc.tensor.matmul(po[:], ones1[:], bb2v[:], start=True, stop=False)
            nc.tensor.matmul(po[:], cT[3][:], tvb2t[:], start=False,
                             stop=False)
            nc.tensor.matmul(po[:], selt[:], s2[:], start=False, stop=True)
            outp = small.tile([B, OJ], f32, name=f"outp_{_rep}", tag="outp")
            nc.vector.tensor_copy(outp[:], po[:])
            nc.scalar.dma_start(out=out_h.ap(), in_=outp[:])

    nc.compile()
    return nc


def _prep_inputs(x, gW1, gb1, gW2, gb2, bW1, bb1, bW2, bb2,
                 tvW1, tvb1, tvW2, tvb2, cfg="mx8"):
    """Build the 8 per-core in_maps (DMA-friendly layouts)."""
    import ml_dtypes

    f = np.float32
    w = np.dtype(ml_dtypes.bfloat16)
    if cfg == "mx8":
        tvd = np.dtype(ml_dtypes.float8_e3m4)
        stv = np.float32(STV)
        q = lambda a: np.clip(a * stv, -15.5, 15.5).astype(tvd)
        gw1f, gb1f = GS / STV, GS
    else:
        tvd = w
        stv = np.float32(STV)
        q = lambda a: (a * stv).astype(tvd)
        gw1f, gb1f = 1.0 / STV, 1.0
    asf = lambda a: np.ascontiguousarray(a, dtype=f)
    asw = lambda a: np.ascontiguousarray(a.astype(f), dtype=w)

    xT = asw(x.T.reshape(KC1, 128, B).transpose(1, 0, 2))
    gw1 = np.ascontiguousarray(
        q(gW1.T.reshape(KC1, 128, D).transpose(1, 0, 2) * gw1f))
    gw2 = np.ascontiguousarray(
        q(gW2.T.reshape(KC1, 128, E * L).transpose(1, 0, 2) * gw1f))
    gb1v = asf(gb1.reshape(1, D) * gb1f)
    gb2v = asf(gb2.reshape(1, E * L) * gb1f)
    sel = np.zeros((128, B), f)
    for j in range(NSTRIP):
        for b in range(B):
            sel[32 * j + b, b] = 1.0 / STV
    sel = asw(sel)

    in_maps = []
    for k in range(NCORES):
        o0 = k * OSL
        j0 = k * OJ
        # L1 bank: [E,128,KC1,OSL] -> quad-grouped [EQUAD,128,4,KC1,OSL]
        tv1 = (tvW1[:, o0:o0 + OSL, :].transpose(0, 2, 1)
               .reshape(E, KC1, 128, OSL).transpose(0, 2, 1, 3))
        tv1 = q(tv1)
        tv1b = np.ascontiguousarray(
            tv1[12:].reshape(2, 2, 128, KC1, OSL).transpose(0, 2, 1, 3, 4))
        tv1 = np.ascontiguousarray(
            tv1[:12].reshape(EQUAD - 1, 4, 128, KC1, OSL)
            .transpose(0, 2, 1, 3, 4))
        bw1 = asw(bW1[o0:o0 + OSL, :].T.reshape(KC1, 128, OSL)
                  .transpose(1, 0, 2) * stv)
        # L2 bank sharded by output column j: tv2[e,p,fc,j] =
        #   tvW2[e, j0+j, 128*fc+p]
        tv2 = (tvW2[:, j0:j0 + OJ, :].transpose(0, 2, 1)
               .reshape(E, KCF, 128, OJ).transpose(0, 2, 1, 3))
        tv2 = np.ascontiguousarray(
            q(tv2.reshape(EQUAD, 4, 128, KCF, OJ).transpose(0, 2, 1, 3, 4)))
        bw2 = asw(bW2[j0:j0 + OJ, :].T.reshape(KCF, 128, OJ)
                  .transpose(1, 0, 2) * stv)
        in_maps.append(dict(
            xT=xT, gw1=gw1, gb1v=gb1v, gw2=gw2, gb2v=gb2v, sel=sel,
            tv1=tv1, tv1b=tv1b, bw1=bw1,
            bb1v=asf(bb1[o0:o0 + OSL].reshape(1, OSL)),
            tvb1=asf(tvb1[:, o0:o0 + OSL]),
            tv2=tv2, bw2=bw2,
            bb2v=asf(bb2[j0:j0 + OJ].reshape(1, OJ)),
            tvb2=asf(tvb2[:, j0:j0 + OJ]),
        ))
    return in_maps


CFG = "mx8"


def kernel(**inputs):
    from concourse.bass_utils import run_bass_kernel_spmd

    key = ("nc", CFG)
    if key not in _cache:
        _cache[key] = _build(cfg=CFG)
    nc = _cache[key]

    in_maps = _prep_inputs(**{k: np.asarray(v) for k, v in inputs.items()},
                           cfg=CFG)
    res = run_bass_kernel_spmd(nc, in_maps, core_ids=list(range(NCORES)))
    return np.concatenate([r["out"] for r in res.results], axis=1)
